# revision 13
# baseline (speedup 1.0000x reference)
"""Trainium2 kernel for the quantum-autoencoder forward pass (nn_AutoEncoder).

Math: the circuit uses only RX and CNOT gates on 8 data qubits (the 2 extra
trash-reference wires and the SWAP-test aux wire stay |0> until measurement).
Conjugating by H^x8 turns every RX into a diagonal RZ and every CNOT into a
basis permutation, so in the X-basis the state is always uniform-magnitude:
psi(x) = (1/16) e^{i theta(x)}, theta(x) = sum_g (t_g/2)(2<m_g,x> - 1) with
GF(2)^8 masks m_g evolved through the CNOT network.

The SWAP test gives p1 = (1 - P00)/2 with P00 = prob(trash wires 6,7 = |00>),
and in the X-frame P00 = (1/4)[1 + sum_{e in {e6,e7,e6^e7}} (1/256) *
sum_x cos(D_e(x))], D_e(x) = sum_{g:<m_g,e>=1} t_g (1 - 2<m_g,x>).

Flattened: p1[b] = 3/8 - (1/2048) * sum_{j<768} cos((A @ f_b)_j + (Pw @ w)_j)
with constant sign matrices A (768x8), Pw (768x32) from the circuit wiring.

Structure exploited on device: up to global row sign (cos is even) the 768
rows of [A|Pw] collapse to 384 distinct rows, each with multiplicity 2, and
those share only 12 distinct A-patterns. With U_k = (A_dist f)_k and
ctil_r = (Pw_dist w)_r:
    sum_j cos(D_j) = 2 * sum_k [ C_k cos(U_k) - S_k sin(U_k) ],
    C_k = sum_{r in grp k} cos(ctil_r),  S_k = sum_{r in grp k} sin(ctil_r)
so per batch row only 24 table lookups (12 sin + 12 cos) are needed; the
C/S weights are computed once per call from the 32 weights.

sin/cos evaluation: the scalar-engine Sin table is only valid on [-pi, pi],
so U is computed in *turns* (A_dist scaled by 1/2pi; cos columns get +0.25
turn) and range-reduced with the fp32 magic-number rounding trick:
t = V + 1.5*2^23 rounds V to the nearest integer k in the upper bits;
mr = (t - M) - V = k - V in [-0.5, 0.5]; sin(2pi V) = sin(-2pi * mr).

Device layout (per core, 512 batch rows, pure data parallel on 8 cores):
batch lives on the FREE axis, the 24 sin/cos terms on partitions. One K=9
matmul produces V^T [24, 512] for the whole shard, two DVE ops range-reduce,
one Sin activation evaluates all terms, and the weighted sum over the 24
terms is a K=24 matmul against the [-S | C] column -> [1, 512] results.
"""

import math
from contextlib import ExitStack

import numpy as np

import concourse.bass as bass
import concourse.tile as tile
from concourse import bacc, mybir
from concourse import masks as cmasks
from concourse.bass_utils import run_bass_kernel_spmd

N_QUBITS = 8
DEPTH = 4
NW = DEPTH * N_QUBITS             # 32 weight angles
BATCH = 4096
N_CORES = 8
SHARD = BATCH // N_CORES          # 512 rows per core
P = 128                           # SBUF partitions
GROUPS = SHARD // P               # 4 batch groups of 128 per core
F32 = mybir.dt.float32
MAGIC = float(1.5 * 2**23)        # fp32 round-to-nearest-integer constant
TWO_PI_GUARD = 2.0 * math.pi * (1.0 - 2.0**-21)  # keep sin arg inside (-pi, pi)


def _build_raw_tables():
    """Phase-tracking masks for the fixed circuit -> sign matrices A, Pw."""
    gates = []  # [mask, ('f'|'w', index)]
    for w in range(N_QUBITS):
        gates.append([1 << w, ("f", w)])
    for l in range(DEPTH):
        for w in range(N_QUBITS):
            gates.append([1 << w, ("w", l * N_QUBITS + w)])
        for w in range(N_QUBITS):
            # original CNOT(ctrl=w, tgt=w+1) -> X-frame ctrl=w+1, tgt=w:
            # masks with bit w set get bit (w+1)%8 flipped
            t, c = w, (w + 1) % N_QUBITS
            for g in gates:
                if g[0] & (1 << t):
                    g[0] ^= 1 << c
    par = np.array([bin(i).count("1") & 1 for i in range(256)], np.int64)
    variants = [1 << 6, 1 << 7, (1 << 6) | (1 << 7)]
    A = np.zeros((3 * 256, N_QUBITS), np.float64)
    Pw = np.zeros((3 * 256, NW), np.float64)
    x = np.arange(256)
    for vi, e in enumerate(variants):
        rows = slice(vi * 256, (vi + 1) * 256)
        for m, (kind, idx) in gates:
            if par[m & e]:
                sigma = 1.0 - 2.0 * par[m & x]
                if kind == "f":
                    A[rows, idx] += sigma
                else:
                    Pw[rows, idx] += sigma
    return A, Pw


def _build_tables():
    A, Pw = _build_raw_tables()
    AB = np.concatenate([A, Pw], axis=1)  # (768, 40)
    # canonicalize row sign by leading nonzero (always in the A part)
    canon = []
    for r in AB:
        nz = np.nonzero(r)[0]
        s = 1.0 if r[nz[0]] > 0 else -1.0
        canon.append(tuple((s * r).tolist()))
    uniq = {}
    for c in canon:
        uniq[c] = uniq.get(c, 0) + 1
    assert len(uniq) == 384 and all(v == 2 for v in uniq.values())
    rows = np.array(list(uniq.keys()))          # (384, 40)
    a_rows = rows[:, :N_QUBITS]                 # (384, 8)
    pw_rows = rows[:, N_QUBITS:]                # (384, 32)
    a_uniq = {}
    for ar in map(tuple, a_rows):
        if ar not in a_uniq:
            a_uniq[ar] = len(a_uniq)
    K = len(a_uniq)
    assert K == 12
    grp = np.array([a_uniq[tuple(ar)] for ar in a_rows])  # (384,)
    a_dist = np.array(list(a_uniq.keys()))                # (12, 8)

    # AD2 (9, 24): contraction rows = 8 feature rows + 1 ones row.
    # cols 0:12 -> U_k in turns, cols 12:24 -> U_k + 0.25 turns
    ad2 = np.zeros((N_QUBITS + 1, 2 * K), np.float64)
    ad2[:N_QUBITS, :K] = a_dist.T / (2 * math.pi)
    ad2[:N_QUBITS, K:] = a_dist.T / (2 * math.pi)
    ad2[N_QUBITS, K:] = 0.25
    # PWD (32, 384): ctil_r = (PWD.T @ w)_r in radians
    pwd = pw_rows.T
    # GM3 (128, 6*24): aggregation weights producing the [-S | C] column
    # directly.  Six K=128 matmuls accumulate wv[0:24,0]; matmul j uses
    # lhsT = GM3[:, 24j:24j+24] with rhs = sinc col j (j<3) / cosc col j-3.
    # wv[k] = -S_k for k<12 (so sin chunks get -1 in rows grp==k) and
    # wv[12+k] = +C_k (cos chunks get +1 in rows 12+k).
    gm3 = np.zeros((P, 6 * 2 * K), np.float64)
    for r in range(384):
        cc, p, k = r // P, r % P, grp[r]
        gm3[p, 2 * K * cc + k] = -1.0              # sin chunk cc -> -S_k
        gm3[p, 2 * K * (3 + cc) + K + k] = 1.0     # cos chunk cc -> +C_k
    return (
        ad2.astype(np.float32),
        np.ascontiguousarray(pwd.astype(np.float32)),
        gm3.astype(np.float32),
        K,
    )


_AD2, _PWD, _GM3, _K = _build_tables()

_CACHE = {}


def _build_nc():
    K = _K
    NT = 2 * K  # 24 sin/cos terms
    nc = bacc.Bacc(
        "TRN2",
        target_bir_lowering=False,
        debug=False,
        num_devices=N_CORES,
    )
    feat = nc.dram_tensor("features", [SHARD, N_QUBITS], F32, kind="ExternalInput")
    wts = nc.dram_tensor("weights", [NW, 1], F32, kind="ExternalInput")
    out = nc.dram_tensor("out", [1, SHARD], F32, kind="ExternalOutput")
    ad2_d = nc.inline_tensor(_AD2, name="tabAD2")  # (9, 24)
    pwd_d = nc.inline_tensor(_PWD, name="tabPWD")  # (32, 384)
    gm_d = nc.inline_tensor(_GM3, name="tabGM3")   # (128, 144)

    SIN = mybir.ActivationFunctionType.Sin
    COPY = mybir.ActivationFunctionType.Copy
    SUB = mybir.AluOpType.subtract

    with tile.TileContext(nc) as tc, ExitStack() as ctx:
        const = ctx.enter_context(tc.tile_pool(name="const", bufs=1))
        work = ctx.enter_context(tc.tile_pool(name="work", bufs=2))
        sps = ctx.enter_context(tc.tile_pool(name="spsum", bufs=1, space="PSUM"))
        tps = ctx.enter_context(tc.tile_pool(name="tpsum", bufs=2, space="PSUM"))
        vps = ctx.enter_context(tc.tile_pool(name="vpsum", bufs=1, space="PSUM"))

        # dummy Sin first: triggers the ACT table load at t=0 so it overlaps
        # the input DMAs instead of sitting on the critical path
        one1 = const.tile([1, 1], F32)
        nc.gpsimd.memset(one1[:], 1.0)
        dummy = const.tile([1, 1], F32)
        nc.scalar.activation(dummy[:], one1[:], SIN, bias=one1[:], scale=0.0)

        ad2_s = const.tile([N_QUBITS + 1, NT], F32)
        nc.sync.dma_start(ad2_s[:], ad2_d.ap()[:])
        pwd_s = const.tile([NW, 3 * P], F32)
        nc.sync.dma_start(pwd_s[:], pwd_d.ap()[:])
        gm_s = const.tile([P, 6 * NT], F32)
        nc.sync.dma_start(gm_s[:], gm_d.ap()[:])
        w_s = const.tile([NW, 1], F32)
        nc.sync.dma_start(w_s[:], wts.ap()[:])
        # features (512,8) -> [128, 32]; partition p, cols 8g:8g+8 hold
        # batch row 128g + p
        f_s = const.tile([P, GROUPS * N_QUBITS], F32)
        nc.sync.dma_start(
            f_s[:].rearrange("p (g w) -> p g w", g=GROUPS),
            feat.ap().rearrange("(g p) w -> p g w", p=P),
        )

        ident = const.tile([P, P], F32)
        cmasks.make_identity(nc, ident[:])
        zeros = const.tile([P, 1], F32)
        nc.gpsimd.memset(zeros[:], 0.0)
        halfpi = const.tile([P, 1], F32)
        nc.gpsimd.memset(halfpi[:], math.pi / 2)

        # ---- per-call setup from weights: wv column [-S | C] (24, 1)
        ct_p = sps.tile([P, 3], F32, tag="setup")  # ctil, 3 chunks of 128
        for cc in range(3):
            nc.tensor.matmul(
                ct_p[:, cc : cc + 1], pwd_s[:, P * cc : P * (cc + 1)], w_s[:],
                start=True, stop=True,
            )
        sinc = const.tile([P, 3], F32)
        nc.scalar.activation(sinc[:], ct_p[:], SIN, bias=zeros[:], scale=1.0)
        cosc = const.tile([P, 3], F32)
        nc.scalar.activation(cosc[:], ct_p[:], SIN, bias=halfpi[:], scale=-1.0)
        wv_p = sps.tile([NT, 1], F32, tag="setup2")
        for j in range(6):
            sc, cc = (sinc, j) if j < 3 else (cosc, j - 3)
            nc.tensor.matmul(
                wv_p[:], gm_s[:, NT * j : NT * (j + 1)], sc[:, cc : cc + 1],
                start=(j == 0), stop=(j == 5),
            )
        wv_col = const.tile([NT, 1], F32)
        nc.vector.tensor_copy(wv_col[:], wv_p[:])

        # ---- main pass: all 512 batch rows at once
        # ftall[w, 128g+p] = features[128g+p, w]; row 8 = 1.0 (bias row)
        ftall = const.tile([N_QUBITS + 1, SHARD], F32)
        nc.gpsimd.memset(ftall[:], 1.0)
        ft_p = tps.tile([N_QUBITS, SHARD], F32)
        for g in range(GROUPS):
            nc.tensor.transpose(
                ft_p[:, P * g : P * (g + 1)],
                f_s[:, N_QUBITS * g : N_QUBITS * (g + 1)],
                ident[:],
            )
        nc.vector.tensor_copy(ftall[:N_QUBITS, :], ft_p[:])

        v_p = vps.tile([NT, SHARD], F32, tag="v")
        nc.tensor.matmul(v_p[:], ad2_s[:], ftall[:], start=True, stop=True)
        t_s = work.tile([NT, SHARD], F32, tag="t")
        nc.vector.tensor_scalar_add(t_s[:], v_p[:], MAGIC)
        mr_s = work.tile([NT, SHARD], F32, tag="mr")
        nc.vector.scalar_tensor_tensor(
            mr_s[:], t_s[:], MAGIC, v_p[:], op0=SUB, op1=SUB
        )
        sv_s = work.tile([NT, SHARD], F32, tag="sv")
        nc.scalar.activation(
            sv_s[:], mr_s[:], SIN, bias=zeros[:NT], scale=-TWO_PI_GUARD
        )
        p_p = vps.tile([1, SHARD], F32, tag="p")
        nc.tensor.matmul(p_p[:], wv_col[:], sv_s[:], start=True, stop=True)
        res = const.tile([1, SHARD], F32)
        nc.scalar.activation(res[:], p_p[:], COPY, bias=3.0 / 8.0, scale=-1.0 / 1024.0)
        nc.sync.dma_start(out.ap()[:], res[:])

    nc.compile()
    return nc


def get_nc():
    if "nc" not in _CACHE:
        _CACHE["nc"] = _build_nc()
    return _CACHE["nc"]


def kernel(features: np.ndarray, weights: np.ndarray, **run_kwargs) -> np.ndarray:
    nc = get_nc()
    feats = np.ascontiguousarray(features, np.float32).reshape(N_CORES, SHARD, N_QUBITS)
    w_col = np.ascontiguousarray(weights, np.float32).reshape(NW, 1)
    in_maps = [{"features": feats[i], "weights": w_col} for i in range(N_CORES)]
    r = run_bass_kernel_spmd(nc, in_maps, core_ids=list(range(N_CORES)), **run_kwargs)
    out = np.concatenate(
        [np.asarray(r.results[i]["out"]).reshape(SHARD) for i in range(N_CORES)]
    )
    if run_kwargs:
        return out.astype(np.float32), r
    return out.astype(np.float32)


# revision 15
# speedup vs baseline: 1.3579x; 1.3579x over previous
"""Trainium2 kernel for the quantum-autoencoder forward pass (nn_AutoEncoder).

Math: the circuit uses only RX and CNOT gates on 8 data qubits (the 2 extra
trash-reference wires and the SWAP-test aux wire stay |0> until measurement).
Conjugating by H^x8 turns every RX into a diagonal RZ and every CNOT into a
basis permutation, so in the X-basis the state is always uniform-magnitude:
psi(x) = (1/16) e^{i theta(x)}, theta(x) = sum_g (t_g/2)(2<m_g,x> - 1) with
GF(2)^8 masks m_g evolved through the CNOT network.

The SWAP test gives p1 = (1 - P00)/2 with P00 = prob(trash wires 6,7 = |00>),
and in the X-frame P00 = (1/4)[1 + sum_{e in {e6,e7,e6^e7}} (1/256) *
sum_x cos(D_e(x))], D_e(x) = sum_{g:<m_g,e>=1} t_g (1 - 2<m_g,x>).

Flattened: p1[b] = 3/8 - (1/2048) * sum_{j<768} cos((A @ f_b)_j + (Pw @ w)_j)
with constant sign matrices A (768x8), Pw (768x32) from the circuit wiring.

Structure exploited on device: up to global row sign (cos is even) the 768
rows of [A|Pw] collapse to 384 distinct rows, each with multiplicity 2, and
those share only 12 distinct A-patterns. With U_k = (A_dist f)_k and
ctil_r = (Pw_dist w)_r:
    sum_j cos(D_j) = 2 * sum_k [ C_k cos(U_k) - S_k sin(U_k) ],
    C_k = sum_{r in grp k} cos(ctil_r),  S_k = sum_{r in grp k} sin(ctil_r)
so per batch row only 24 table lookups (12 sin + 12 cos) are needed; the
C/S weights are computed once per call from the 32 weights.

sin/cos evaluation: the scalar-engine Sin table is only valid on [-pi, pi],
so U is computed in *turns* (A_dist scaled by 1/2pi; cos columns get +0.25
turn) and range-reduced with the fp32 magic-number rounding trick:
t = V + 1.5*2^23 rounds V to the nearest integer k in the upper bits;
mr = (t - M) - V = k - V in [-0.5, 0.5]; sin(2pi V) = sin(-2pi * mr).

Device layout (per core, 512 batch rows, pure data parallel on 8 cores):
batch lives on the FREE axis, the 24 sin/cos terms on partitions. One K=9
matmul produces V^T [24, 512] for the whole shard, two DVE ops range-reduce,
one Sin activation evaluates all terms, and the weighted sum over the 24
terms is a K=24 matmul against the [-S | C] column -> [1, 512] results.
"""

import math
from contextlib import ExitStack

import numpy as np

import concourse.bass as bass
import concourse.tile as tile
from concourse import bacc, mybir
from concourse import masks as cmasks
from concourse.bass_utils import run_bass_kernel_spmd

N_QUBITS = 8
DEPTH = 4
NW = DEPTH * N_QUBITS             # 32 weight angles
BATCH = 4096
N_CORES = 8
SHARD = BATCH // N_CORES          # 512 rows per core
P = 128                           # SBUF partitions
GROUPS = SHARD // P               # 4 batch groups of 128 per core
F32 = mybir.dt.float32
MAGIC = float(1.5 * 2**23)        # fp32 round-to-nearest-integer constant
TWO_PI_GUARD = 2.0 * math.pi * (1.0 - 2.0**-21)  # keep sin arg inside (-pi, pi)


def _build_raw_tables():
    """Phase-tracking masks for the fixed circuit -> sign matrices A, Pw."""
    gates = []  # [mask, ('f'|'w', index)]
    for w in range(N_QUBITS):
        gates.append([1 << w, ("f", w)])
    for l in range(DEPTH):
        for w in range(N_QUBITS):
            gates.append([1 << w, ("w", l * N_QUBITS + w)])
        for w in range(N_QUBITS):
            # original CNOT(ctrl=w, tgt=w+1) -> X-frame ctrl=w+1, tgt=w:
            # masks with bit w set get bit (w+1)%8 flipped
            t, c = w, (w + 1) % N_QUBITS
            for g in gates:
                if g[0] & (1 << t):
                    g[0] ^= 1 << c
    par = np.array([bin(i).count("1") & 1 for i in range(256)], np.int64)
    variants = [1 << 6, 1 << 7, (1 << 6) | (1 << 7)]
    A = np.zeros((3 * 256, N_QUBITS), np.float64)
    Pw = np.zeros((3 * 256, NW), np.float64)
    x = np.arange(256)
    for vi, e in enumerate(variants):
        rows = slice(vi * 256, (vi + 1) * 256)
        for m, (kind, idx) in gates:
            if par[m & e]:
                sigma = 1.0 - 2.0 * par[m & x]
                if kind == "f":
                    A[rows, idx] += sigma
                else:
                    Pw[rows, idx] += sigma
    return A, Pw


def _build_tables():
    A, Pw = _build_raw_tables()
    AB = np.concatenate([A, Pw], axis=1)  # (768, 40)
    # canonicalize row sign by leading nonzero (always in the A part)
    canon = []
    for r in AB:
        nz = np.nonzero(r)[0]
        s = 1.0 if r[nz[0]] > 0 else -1.0
        canon.append(tuple((s * r).tolist()))
    uniq = {}
    for c in canon:
        uniq[c] = uniq.get(c, 0) + 1
    assert len(uniq) == 384 and all(v == 2 for v in uniq.values())
    rows = np.array(list(uniq.keys()))          # (384, 40)
    a_rows = rows[:, :N_QUBITS]                 # (384, 8)
    pw_rows = rows[:, N_QUBITS:]                # (384, 32)
    a_uniq = {}
    for ar in map(tuple, a_rows):
        if ar not in a_uniq:
            a_uniq[ar] = len(a_uniq)
    K = len(a_uniq)
    assert K == 12
    grp = np.array([a_uniq[tuple(ar)] for ar in a_rows])  # (384,)
    a_dist = np.array(list(a_uniq.keys()))                # (12, 8)

    # AD2 (9, 24): contraction rows = 8 feature rows + 1 ones row.
    # cols 0:12 -> U_k in turns, cols 12:24 -> U_k + 0.25 turns
    ad2 = np.zeros((N_QUBITS + 1, 2 * K), np.float64)
    ad2[:N_QUBITS, :K] = a_dist.T / (2 * math.pi)
    ad2[:N_QUBITS, K:] = a_dist.T / (2 * math.pi)
    ad2[N_QUBITS, K:] = 0.25
    # PWD (32, 384): ctil_r = (PWD.T @ w)_r in radians
    pwd = pw_rows.T
    # GM3 (128, 6*24): aggregation weights producing the [-S | C] column
    # directly.  Six K=128 matmuls accumulate wv[0:24,0]; matmul j uses
    # lhsT = GM3[:, 24j:24j+24] with rhs = sinc col j (j<3) / cosc col j-3.
    # wv[k] = -S_k for k<12 (so sin chunks get -1 in rows grp==k) and
    # wv[12+k] = +C_k (cos chunks get +1 in rows 12+k).
    gm3 = np.zeros((P, 6 * 2 * K), np.float64)
    for r in range(384):
        cc, p, k = r // P, r % P, grp[r]
        gm3[p, 2 * K * cc + k] = -1.0              # sin chunk cc -> -S_k
        gm3[p, 2 * K * (3 + cc) + K + k] = 1.0     # cos chunk cc -> +C_k
    return (
        ad2.astype(np.float32),
        np.ascontiguousarray(pwd.astype(np.float32)),
        gm3.astype(np.float32),
        K,
    )


_AD2, _PWD, _GM3, _K = _build_tables()


def _packed_consts():
    """One [128, 24+384+144] constant block -> a single DMA.
    cols 0:24 = AD2 (rows 0:9), 24:408 = PWD (rows 0:32), 408:552 = GM3."""
    pk = np.zeros((P, 2 * _K + 3 * P + 12 * _K), np.float32)
    pk[: N_QUBITS + 1, : 2 * _K] = _AD2
    pk[:NW, 2 * _K : 2 * _K + 3 * P] = _PWD
    pk[:, 2 * _K + 3 * P :] = _GM3
    return np.ascontiguousarray(pk)


_PK = _packed_consts()

_CACHE = {}


def _build_nc():
    K = _K
    NT = 2 * K  # 24 sin/cos terms
    nc = bacc.Bacc(
        "TRN2",
        target_bir_lowering=False,
        debug=False,
        num_devices=N_CORES,
    )
    feat = nc.dram_tensor("features", [SHARD, N_QUBITS], F32, kind="ExternalInput")
    wts = nc.dram_tensor("weights", [NW, 1], F32, kind="ExternalInput")
    out = nc.dram_tensor("out", [1, SHARD], F32, kind="ExternalOutput")
    pk_d = nc.inline_tensor(_PK, name="tabPK")  # (128, 552) packed consts

    SIN = mybir.ActivationFunctionType.Sin
    COPY = mybir.ActivationFunctionType.Copy
    SUB = mybir.AluOpType.subtract

    with tile.TileContext(nc) as tc, ExitStack() as ctx:
        const = ctx.enter_context(tc.tile_pool(name="const", bufs=1))
        work = ctx.enter_context(tc.tile_pool(name="work", bufs=2))
        sps = ctx.enter_context(tc.tile_pool(name="spsum", bufs=1, space="PSUM"))
        tps = ctx.enter_context(tc.tile_pool(name="tpsum", bufs=2, space="PSUM"))
        vps = ctx.enter_context(tc.tile_pool(name="vpsum", bufs=1, space="PSUM"))

        # dummy Sin first: triggers the ACT table load at t=0 so it overlaps
        # the input DMAs instead of sitting on the critical path
        one1 = const.tile([1, 1], F32)
        nc.gpsimd.memset(one1[:], 1.0)
        dummy = const.tile([1, 1], F32)
        nc.scalar.activation(dummy[:], one1[:], SIN, bias=one1[:], scale=0.0)

        # DMAs in critical-path order: features gate the main chain, then
        # weights + packed consts for the setup chain
        # features (512,8) -> [128, 32]; partition p, cols 8g:8g+8 hold
        # batch row 128g + p
        f_s = const.tile([P, GROUPS * N_QUBITS], F32)
        nc.sync.dma_start(
            f_s[:].rearrange("p (g w) -> p g w", g=GROUPS),
            feat.ap().rearrange("(g p) w -> p g w", p=P),
        )
        w_s = const.tile([NW, 1], F32)
        nc.sync.dma_start(w_s[:], wts.ap()[:])
        pk_s = const.tile([P, _PK.shape[1]], F32)
        nc.sync.dma_start(pk_s[:], pk_d.ap()[:])
        ad2_s = pk_s[: N_QUBITS + 1, :NT]
        pwd_s = pk_s[:NW, NT : NT + 3 * P]
        gm_s = pk_s[:, NT + 3 * P :]

        ident = const.tile([P, P], F32)
        cmasks.make_identity(nc, ident[:])
        zeros = const.tile([P, 1], F32)
        nc.gpsimd.memset(zeros[:], 0.0)
        halfpi = const.tile([P, 1], F32)
        nc.gpsimd.memset(halfpi[:], math.pi / 2)
        # ftall[w, 128g+p] = features[128g+p, w]; row 8 = 1.0 (bias row)
        ftall = const.tile([N_QUBITS + 1, SHARD], F32)
        nc.gpsimd.memset(ftall[:], 1.0)

        # ---- main chain first (engine-queue order = readiness order)
        ft_p = tps.tile([N_QUBITS, SHARD], F32)
        for g in range(GROUPS):
            nc.tensor.transpose(
                ft_p[:, P * g : P * (g + 1)],
                f_s[:, N_QUBITS * g : N_QUBITS * (g + 1)],
                ident[:],
            )
        nc.vector.tensor_copy(ftall[:N_QUBITS, :], ft_p[:])
        v_p = vps.tile([NT, SHARD], F32, tag="v")
        nc.tensor.matmul(v_p[:], ad2_s, ftall[:], start=True, stop=True)
        t_s = work.tile([NT, SHARD], F32, tag="t")
        nc.vector.tensor_scalar_add(t_s[:], v_p[:], MAGIC)
        mr_s = work.tile([NT, SHARD], F32, tag="mr")
        nc.vector.scalar_tensor_tensor(
            mr_s[:], t_s[:], MAGIC, v_p[:], op0=SUB, op1=SUB
        )

        # ---- weight setup (off critical path; ACT before the big Sin)
        ct_p = sps.tile([P, 3], F32, tag="setup")  # ctil, 3 chunks of 128
        for cc in range(3):
            nc.tensor.matmul(
                ct_p[:, cc : cc + 1], pwd_s[:, P * cc : P * (cc + 1)], w_s[:],
                start=True, stop=True,
            )
        sinc = const.tile([P, 3], F32)
        nc.scalar.activation(sinc[:], ct_p[:], SIN, bias=zeros[:], scale=1.0)
        cosc = const.tile([P, 3], F32)
        nc.scalar.activation(cosc[:], ct_p[:], SIN, bias=halfpi[:], scale=-1.0)
        wv_p = sps.tile([NT, 1], F32, tag="setup2")
        for j in range(6):
            sc, cc = (sinc, j) if j < 3 else (cosc, j - 3)
            nc.tensor.matmul(
                wv_p[:], gm_s[:, NT * j : NT * (j + 1)], sc[:, cc : cc + 1],
                start=(j == 0), stop=(j == 5),
            )
        wv_col = const.tile([NT, 1], F32)
        nc.vector.tensor_copy(wv_col[:], wv_p[:])

        # ---- tail of the main chain
        sv_s = work.tile([NT, SHARD], F32, tag="sv")
        nc.scalar.activation(
            sv_s[:], mr_s[:], SIN, bias=zeros[:NT], scale=-TWO_PI_GUARD
        )
        p_p = vps.tile([1, SHARD], F32, tag="p")
        nc.tensor.matmul(p_p[:], wv_col[:], sv_s[:], start=True, stop=True)
        res = const.tile([1, SHARD], F32)
        nc.scalar.activation(res[:], p_p[:], COPY, bias=3.0 / 8.0, scale=-1.0 / 1024.0)
        nc.sync.dma_start(out.ap()[:], res[:])

    nc.compile()
    return nc


def get_nc():
    if "nc" not in _CACHE:
        _CACHE["nc"] = _build_nc()
    return _CACHE["nc"]


def kernel(features: np.ndarray, weights: np.ndarray, **run_kwargs) -> np.ndarray:
    nc = get_nc()
    feats = np.ascontiguousarray(features, np.float32).reshape(N_CORES, SHARD, N_QUBITS)
    w_col = np.ascontiguousarray(weights, np.float32).reshape(NW, 1)
    in_maps = [{"features": feats[i], "weights": w_col} for i in range(N_CORES)]
    r = run_bass_kernel_spmd(nc, in_maps, core_ids=list(range(N_CORES)), **run_kwargs)
    out = np.concatenate(
        [np.asarray(r.results[i]["out"]).reshape(SHARD) for i in range(N_CORES)]
    )
    if run_kwargs:
        return out.astype(np.float32), r
    return out.astype(np.float32)
